# revision 5
# baseline (speedup 1.0000x reference)
"""Trainium2 Bass kernel v2 for the pairwise-classifier loss.

Math (same reduction as v1): per branch, logits = x @ W + b with 2 classes
reduces to d = lo[e1] + hi[e2] + db, y = tanh(d/2), row losses
loss_neg = softplus(y), loss_pos = softplus(y) - y.

v2 design changes vs v1:
  - replicated table build (no collective, no cross-core skew wait):
    every core loads all 16MB of features (f32->bf16 cast DMA, per-b
    pipelined), projects to per-branch tables via PE, writes bf16
    tables to DRAM, and broadcasts them to SBUF per-b as soon as each
    b's chunk lands (overlaps PE with broadcast DMA).
  - branch-routed Q7 cores: cores 0-3 (partitions 0-63) process the
    row branch, cores 4-7 the col branch.  Each partition then only
    needs its own branch's 64KB table -> 8MB broadcast instead of 16MB.
  - 3 ap_gathers (e1, e3, e2) of num_idxs=3200 instead of 6 of 1600.
  - gather offsets are computed on the host and uploaded as int16.
  - softplus activation fuses the exp+ln chain (2 ACT ops per row set).

Table row (branch-local): rb(b, n) = b*2048 + (n%128)*16 + (n//128),
so core-computed psum columns land contiguously.
"""

import numpy as np

import concourse.bass as bass
import concourse.bacc as bacc
import concourse.mybir as mybir
import concourse.tile as tile
from concourse.bass_utils import run_bass_kernel_spmd

F32 = mybir.dt.float32
BF16 = mybir.dt.bfloat16
I16 = mybir.dt.int16

B, C, N = 8, 256, 2048
R = 100000
NCORES = 8
PAIRS = R // NCORES          # 12500 pairs per core per branch
P = 128
QC = 4                       # Q7 cores per branch half
PPQ = PAIRS // QC            # 3125 real pairs per Q7 core
NI = 3128                    # ap_gather num_idxs per core (3125 + 3 pad)
GK = 196                     # int16 idx words per channel (ceil(NI/16))
# wave-1: the first W1 slots of every core hold pairs whose endpoints all
# have b <= 5, so their gathers only read the table prefix written by the
# first 12 broadcasts and can start ~25us before the full table is ready.
W1 = 512
BPREF = 4                    # wave-1 table prefix covers b < BPREF
# e2 chunks; each chunk's chains hide under the next chunk's gather.
E2_CHUNKS = ((0, W1), (W1, 1440), (1952, 656), (2608, 520))
TROWS = B * N                # 16384 rows per branch table
PADSLOT = 3126               # a known pad list position (>= PPQ, in the tail chunk)


def _emit_weight_prep(nc, const, psmall, w_row, w_col, b_row, b_col):
    """wp [128, 2kt*4m] bf16 (m = row_lo,row_hi,col_lo,col_hi) and
    dbp [128, 1] f32 = db/2 per partition (row for p<64, col for p>=64)."""
    wr_raw = const.tile([P, 8], F32, tag="wr_raw")
    wc_raw = const.tile([P, 8], F32, tag="wc_raw")
    nc.sync.dma_start(
        out=wr_raw[:].rearrange("p (s t) -> p s t", s=4),
        in_=w_row[:].rearrange("(s p) t -> p s t", p=P),
    )
    nc.sync.dma_start(
        out=wc_raw[:].rearrange("p (s t) -> p s t", s=4),
        in_=w_col[:].rearrange("(s p) t -> p s t", p=P),
    )
    wdiff_r = const.tile([P, 4], F32, tag="wdiff_r")
    wdiff_c = const.tile([P, 4], F32, tag="wdiff_c")
    nc.vector.tensor_tensor(
        out=wdiff_r[:], in0=wr_raw[:, 1::2], in1=wr_raw[:, 0::2],
        op=mybir.AluOpType.subtract,
    )
    nc.vector.tensor_tensor(
        out=wdiff_c[:], in0=wc_raw[:, 1::2], in1=wc_raw[:, 0::2],
        op=mybir.AluOpType.subtract,
    )
    wp = const.tile([P, 8], BF16, tag="wp")
    nc.vector.tensor_copy(out=wp[:, 0:8:4], in_=wdiff_r[:, 0:2])
    nc.vector.tensor_copy(out=wp[:, 1:8:4], in_=wdiff_r[:, 2:4])
    nc.vector.tensor_copy(out=wp[:, 2:8:4], in_=wdiff_c[:, 0:2])
    nc.vector.tensor_copy(out=wp[:, 3:8:4], in_=wdiff_c[:, 2:4])

    br_raw = const.tile([1, 2], F32, tag="br_raw")
    bc_raw = const.tile([1, 2], F32, tag="bc_raw")
    nc.sync.dma_start(out=br_raw[:], in_=b_row[:])
    nc.sync.dma_start(out=bc_raw[:], in_=b_col[:])
    db_rc = const.tile([1, 2], F32, tag="db_rc")
    nc.vector.tensor_tensor(
        out=db_rc[:, 0:1], in0=br_raw[:, 1:2], in1=br_raw[:, 0:1],
        op=mybir.AluOpType.subtract,
    )
    nc.vector.tensor_tensor(
        out=db_rc[:, 1:2], in0=bc_raw[:, 1:2], in1=bc_raw[:, 0:1],
        op=mybir.AluOpType.subtract,
    )
    # broadcast db/2 to all partitions via a 0.5-valued ones row
    half_row = const.tile([1, P], F32, tag="half_row")
    nc.vector.memset(half_row[:], 0.5)
    db_psum = psmall.tile([P, 2], F32, tag="db_psum")
    nc.tensor.matmul(
        db_psum[:], lhsT=half_row[:], rhs=db_rc[:], start=True, stop=True,
    )
    dbp = const.tile([P, 1], F32, tag="dbp")
    nc.vector.tensor_copy(out=dbp[0:64, :], in_=db_psum[0:64, 0:1])
    nc.vector.tensor_copy(out=dbp[64:P, :], in_=db_psum[64:P, 1:2])
    return wp, dbp


def _build_nc():
    nc = bacc.Bacc()

    feats = nc.declare_dram_parameter("feats", [B, C, N], F32, isOutput=False)
    w_row = nc.declare_dram_parameter("w_row", [2 * C, 2], F32, isOutput=False)
    w_col = nc.declare_dram_parameter("w_col", [2 * C, 2], F32, isOutput=False)
    b_row = nc.declare_dram_parameter("b_row", [1, 2], F32, isOutput=False)
    b_col = nc.declare_dram_parameter("b_col", [1, 2], F32, isOutput=False)
    idx = nc.declare_dram_parameter("idx", [P, 3 * GK], I16, isOutput=False)
    partial = nc.declare_dram_parameter("partial", [1], F32, isOutput=True)

    # t16[b, branch] = 2048 x (lo,hi) bf16 rows, row (n%128)*16 + n//128
    t16 = nc.dram_tensor("t16", [B, 2, N, 2], BF16)

    with tile.TileContext(nc) as tc:
        with (
            tc.tile_pool(name="const", bufs=1) as const,
            tc.tile_pool(name="fbpool", bufs=2) as fbpool,
            tc.tile_pool(name="chunkp", bufs=2) as chunkp,
            tc.tile_pool(name="work", bufs=2) as work,
            tc.tile_pool(name="psum", bufs=2, space="PSUM") as psum,
            tc.tile_pool(name="psmall", bufs=1, space="PSUM") as psmall,
        ):
            wp, dbp = _emit_weight_prep(nc, const, psmall, w_row, w_col,
                                        b_row, b_col)
            idx_sb = const.tile([P, 3 * GK], I16, tag="idx_sb")
            nc.sync.dma_start(out=idx_sb[:], in_=idx[:])

            # branch-split per-partition table: p<64 row table, p>=64 col
            table_sb = const.tile([P, TROWS * 2], BF16, tag="table_sb")

            # feats loads: cast-DMA stream on the gpsimd (SWDGE) queue;
            # chunk writes + broadcasts ride the sync/scalar queues so no
            # compute-dependent stall sits in front of a load.
            sbcs = []
            for b in range(B):
                fb = fbpool.tile([P, 2 * N], BF16, tag="fb", bufs=2)
                if b < B - 2:
                    nc.gpsimd.dma_start(
                        out=fb[:].rearrange("p (kt n) -> p kt n", kt=2),
                        in_=feats[b].rearrange("(kt p) n -> p kt n", p=P),
                    )
                else:
                    # last two b's ride the HWDGE queues as f32 + DVE cast
                    # so the gpsimd cast stream finishes ~20us earlier
                    for kt, eng in ((0, nc.sync), (1, nc.scalar)):
                        fb32 = fbpool.tile([P, N], F32, tag="fb32", bufs=2)
                        eng.dma_start(
                            out=fb32[:],
                            in_=feats[b, kt * P:(kt + 1) * P],
                        )
                        nc.vector.tensor_copy(
                            out=fb[:, kt * N:(kt + 1) * N], in_=fb32[:])
                pt = psum.tile([P, 64], F32, tag="pt", bufs=2)
                for blk in range(16):
                    nc.tensor.matmul(
                        pt[:, blk * 4:(blk + 1) * 4],
                        lhsT=fb[:, blk * P:(blk + 1) * P],
                        rhs=wp[:, 0:4], start=True, stop=False,
                    )
                    nc.tensor.matmul(
                        pt[:, blk * 4:(blk + 1) * 4],
                        lhsT=fb[:, N + blk * P:N + (blk + 1) * P],
                        rhs=wp[:, 4:8], start=False, stop=True,
                    )
                sbc = chunkp.tile([P, 64], BF16, tag="sbc", bufs=8)
                ptv = pt[:].rearrange("p (blk m) -> p blk m", m=4)
                nc.vector.tensor_copy(
                    out=sbc[:, 0:32].rearrange("p (blk t) -> p blk t", t=2),
                    in_=ptv[:, :, 0:2],
                )
                nc.vector.tensor_copy(
                    out=sbc[:, 32:64].rearrange("p (blk t) -> p blk t", t=2),
                    in_=ptv[:, :, 2:4],
                )
                sbcs.append(sbc)
            # table chunk writes trail on the sync queue
            wrs = []
            for b in range(B):
                for h in (0, 1):
                    w_dma = nc.sync.dma_start(
                        out=t16[b, h].rearrange("(q blk) t -> q blk t", q=P),
                        in_=sbcs[b][:, 32 * h:32 * (h + 1)].rearrange(
                            "p (blk t) -> p blk t", t=2),
                    )
                    wrs.append((b, h, w_dma))
            # broadcasts trail on sync (h=0) and scalar (h=1)
            bcast_eng = {0: nc.scalar, 1: nc.scalar}
            for b, h, w_dma in wrs:
                bc = bcast_eng[h].dma_start(
                    out=table_sb[64 * h:64 * (h + 1),
                                 b * 2 * N:(b + 1) * 2 * N],
                    in_=t16[b, h].rearrange(
                        "r t -> (r t)").partition_broadcast(64),
                )
                tile.add_dep_helper(bc.ins, w_dma.ins, sync=True,
                                    reason="broadcast after chunk write")

            # gathers: wave-1 reads only the b<BPREF table prefix
            table_ap = table_sb[:].rearrange("p (e d) -> p e d", d=2)
            pref_ap = table_sb[:, 0:BPREF * 2 * N].rearrange(
                "p (e d) -> p e d", d=2)

            def gather(s, f0w=0, ni=NI, pref=False):
                g_t = work.tile([P, ni, 2], BF16, tag=f"g{s}_{f0w}", bufs=1)
                nw = (ni + 15) // 16
                nc.gpsimd.ap_gather(
                    out_ap=g_t[:],
                    in_ap=pref_ap if pref else table_ap,
                    idxs_ap=idx_sb[:, s * GK + f0w:s * GK + f0w + nw],
                    channels=P,
                    num_elems=BPREF * N if pref else TROWS,
                    d=2, num_idxs=ni,
                )
                return g_t

            acc = const.tile([P, 14], F32, tag="acc")
            bias_one = const.tile([P, 1], F32, tag="bias_one")
            nc.vector.memset(bias_one[:], 1.0)


            def d_add(in0, in1, ni):
                d_t = work.tile([P, ni], BF16, tag=f"d{ni}", bufs=2)
                nc.vector.tensor_tensor(
                    out=d_t[:], in0=in0, in1=in1, op=mybir.AluOpType.add,
                )
                return d_t

            def tanh_op(nm, d_t, ni, acc_y=None):
                y_t = work.tile([P, ni], BF16, tag=f"y{nm}", bufs=1)
                kw = {}
                if acc_y is not None:
                    kw["accum_out"] = acc_y
                nc.scalar.activation(
                    out=y_t[:], in_=d_t[:],
                    func=mybir.ActivationFunctionType.Tanh,
                    bias=dbp[:, 0:1], scale=0.5, **kw,
                )
                return y_t

            def expln_op(nm, y_t, ni, acc_l, e_t):
                # e overwrites the (dead) d tile of the same chain
                nc.scalar.activation(
                    out=e_t[:], in_=y_t[:],
                    func=mybir.ActivationFunctionType.Exp,
                )
                l_t = work.tile([P, ni], BF16, tag=f"l{nm}", bufs=1)
                nc.scalar.activation(
                    out=l_t[:], in_=e_t[:],
                    func=mybir.ActivationFunctionType.Ln,
                    bias=bias_one[:, 0:1], scale=1.0,
                    accum_out=acc_l,
                )
                return l_t

            NI2 = NI - W1
            g1a = gather(0, 0, W1, pref=True)
            g3a = gather(2, 0, W1, pref=True)

            def e2_chunk(ci, c0, cn, g1t, g3t, o0):
                g2 = gather(1, c0 // 16, cn, pref=(ci == 0))
                d_pos_c = d_add(g1t[:, c0 - o0:c0 - o0 + cn, 0],
                                g2[:, :, 1], cn)
                d_negB_c = d_add(g3t[:, c0 - o0:c0 - o0 + cn, 0],
                                 g2[:, :, 1], cn)
                y_pos_c = tanh_op(f"pos{ci}", d_pos_c, cn,
                                  acc_y=acc[:, 10 + ci:11 + ci])
                y_negB_c = tanh_op(f"negB{ci}", d_negB_c, cn)
                l_pos_c = expln_op(f"pos{ci}", y_pos_c, cn,
                                   acc[:, 2 + 2 * ci:3 + 2 * ci], d_pos_c)
                expln_op(f"negB{ci}", y_negB_c, cn,
                         acc[:, 3 + 2 * ci:4 + 2 * ci], d_negB_c)
                return y_pos_c, l_pos_c

            # wave-1 e2 chunk + its chains run while the full table finishes
            e2_chunk(0, 0, W1, g1a, g3a, 0)
            d_negA1 = d_add(g3a[:, :, 0], g1a[:, :, 1], W1)
            y_negA1 = tanh_op("negA1", d_negA1, W1)
            expln_op("negA1", y_negA1, W1, acc[:, 0:1], d_negA1)
            # wave-2 full-table gathers
            g1b = gather(0, W1 // 16, NI2)
            g3b = gather(2, W1 // 16, NI2)
            d_negA2 = d_add(g3b[:, :, 0], g1b[:, :, 1], NI2)
            y_negA2 = tanh_op("negA2", d_negA2, NI2)
            expln_op("negA2", y_negA2, NI2, acc[:, 1:2], d_negA2)
            y_pos = l_pos = None
            for ci, (c0, cn) in enumerate(E2_CHUNKS[1:], start=1):
                y_pos, l_pos = e2_chunk(ci, c0, cn, g1b, g3b, W1)


            # total = sum(L) - sum(y_pos), scaled by 1/(3R*16)
            t_l = const.tile([P, 1], F32, tag="t_l")
            nc.vector.tensor_reduce(
                out=t_l[:], in_=acc[:, 0:10], axis=mybir.AxisListType.X,
                op=mybir.AluOpType.add,
            )
            t_y = const.tile([P, 1], F32, tag="t_y")
            nc.vector.tensor_reduce(
                out=t_y[:], in_=acc[:, 10:14], axis=mybir.AxisListType.X,
                op=mybir.AluOpType.add,
            )
            total = const.tile([P, 1], F32, tag="total")
            nc.vector.tensor_tensor(
                out=total[:], in0=t_l[:], in1=t_y[:],
                op=mybir.AluOpType.subtract,
            )
            scale_vec = const.tile([P, 1], F32, tag="scale_vec")
            nc.vector.memset(scale_vec[:], 1.0 / (3.0 * R * 16.0))
            out_psum = psmall.tile([1, 1], F32, tag="out_psum")
            nc.tensor.matmul(
                out_psum[:], lhsT=total[:], rhs=scale_vec[:],
                start=True, stop=True,
            )
            out_sb = const.tile([1, 1], F32, tag="out_sb")
            nc.vector.tensor_copy(out=out_sb[:], in_=out_psum[:])

            # pad correction: each branch half has 4 cores x 75 pad pairs,
            # all reading table row 0; per pad pair the pollution is
            # 3*L - y, totalling (3L - y)*300/(3R) = 0.003*L - 0.001*y.
            corr = const.tile([1, 4], F32, tag="corr")
            ps = PADSLOT - E2_CHUNKS[-1][0]  # pad slot within last chunk
            for h in (0, 1):
                nc.vector.tensor_copy(
                    out=corr[0:1, h:h + 1],
                    in_=l_pos[64 * h:64 * h + 1, ps:ps + 1])
                nc.vector.tensor_copy(
                    out=corr[0:1, 2 + h:3 + h],
                    in_=y_pos[64 * h:64 * h + 1, ps:ps + 1])
            cs = const.tile([1, 2], F32, tag="cs")
            nc.vector.tensor_tensor(out=cs[0:1, 0:1], in0=corr[0:1, 0:1],
                                    in1=corr[0:1, 1:2],
                                    op=mybir.AluOpType.add)
            nc.vector.tensor_tensor(out=cs[0:1, 1:2], in0=corr[0:1, 2:3],
                                    in1=corr[0:1, 3:4],
                                    op=mybir.AluOpType.add)
            # 3 pads/core x 4 cores = 12 pad pairs per branch:
            # pollution = (3L - y) * 12 / (3R)
            cs2 = const.tile([1, 2], F32, tag="cs2")
            nc.vector.tensor_scalar(out=cs2[0:1, 0:1], in0=cs[0:1, 0:1],
                                    scalar1=3.0 * 12 / (3.0 * R),
                                    scalar2=None,
                                    op0=mybir.AluOpType.mult)
            nc.vector.tensor_scalar(out=cs2[0:1, 1:2], in0=cs[0:1, 1:2],
                                    scalar1=12 / (3.0 * R), scalar2=None,
                                    op0=mybir.AluOpType.mult)
            out2 = const.tile([1, 1], F32, tag="out2")
            nc.vector.tensor_tensor(out=out2[:], in0=out_sb[:],
                                    in1=cs2[0:1, 0:1],
                                    op=mybir.AluOpType.subtract)
            nc.vector.tensor_tensor(out=out2[:], in0=out2[:],
                                    in1=cs2[0:1, 1:2],
                                    op=mybir.AluOpType.add)
            nc.sync.dma_start(out=partial[:], in_=out2[0, :])
    return nc


_NC_CACHE = {}


def _get_nc():
    if "v2" not in _NC_CACHE:
        nc = _build_nc()
        nc.finalize()
        _NC_CACHE["v2"] = nc
    return _NC_CACHE["v2"]


def _pack_core_inputs(inputs, core):
    """Host-side: compute branch-local int16 table offsets and arrange
    them in the (Q7-core wrapped) ap_gather index layout.

    Slot (p, s*GK + F): half h = p//64, core c = (p%64)//16, lane
    u = p%16, list pos j = F*16 + u; pair = c*PPQ + j for j < PPQ,
    else pad (offset 0)."""
    base = core * PAIRS
    sl = slice(base, base + PAIRS)
    branches = [
        (inputs["row_pos_b"][sl], inputs["row_pos_i"][sl],
         inputs["row_pos_j"][sl], inputs["row_neg_b"][sl],
         inputs["row_neg_i"][sl]),
        (inputs["col_pos_b"][sl], inputs["col_pos_i"][sl],
         inputs["col_pos_j"][sl], inputs["col_neg_b"][sl],
         inputs["col_neg_i"][sl]),
    ]

    def off(b, n):
        b = np.asarray(b, np.int64)
        n = np.asarray(n, np.int64)
        return b * 2048 + (n & 127) * 16 + (n >> 7)

    arr = np.zeros((P, 3 * GK), np.int16)
    u = np.arange(16)[:, None]
    F = np.arange(GK)[None, :]
    j = F * 16 + u                      # [16, GK] list positions
    valid = j < PPQ
    jc = np.minimum(j, PPQ - 1)
    for h, (pb, pi, pj, nb, ni_) in enumerate(branches):
        offs = (off(pb, pi), off(pb, pj), off(nb, ni_))
        w1m = (np.asarray(pb) < 4) & (np.asarray(nb) < 4)
        for c in range(QC):
            lo = c * PPQ
            w1c = np.nonzero(w1m[lo:lo + PPQ])[0]
            assert len(w1c) >= W1, len(w1c)
            rest = np.concatenate(
                [w1c[W1:], np.nonzero(~w1m[lo:lo + PPQ])[0]])
            perm = np.concatenate([w1c[:W1], rest]) + lo
            for s in range(3):
                o = offs[s][perm]
                vals = np.where(valid, o[jc], 0)
                arr[h * 64 + c * 16:h * 64 + (c + 1) * 16,
                    s * GK:(s + 1) * GK] = vals.astype(np.int16)

    im = {
        "w_row": np.ascontiguousarray(np.asarray(inputs["W_row"], np.float32)),
        "w_col": np.ascontiguousarray(np.asarray(inputs["W_col"], np.float32)),
        "b_row": np.ascontiguousarray(
            np.asarray(inputs["b_row"], np.float32).reshape(1, 2)),
        "b_col": np.ascontiguousarray(
            np.asarray(inputs["b_col"], np.float32).reshape(1, 2)),
        "idx": arr,
        "feats": np.ascontiguousarray(
            np.asarray(inputs["all_features"], np.float32)),
    }
    return im


def run(inputs, trace=False):
    nc = _get_nc()
    in_maps = [_pack_core_inputs(inputs, c) for c in range(NCORES)]
    res = run_bass_kernel_spmd(nc, in_maps, list(range(NCORES)), trace=trace)
    partials = np.array(
        [res.results[c]["partial"][0] for c in range(NCORES)], np.float32
    )
    out = np.array([partials.sum()], np.float32)
    return out, res


def kernel(**inputs):
    out, _ = run(inputs, trace=False)
    return out


# revision 6
# speedup vs baseline: 1.0135x; 1.0135x over previous
"""Trainium2 Bass kernel v2 for the pairwise-classifier loss.

Math (same reduction as v1): per branch, logits = x @ W + b with 2 classes
reduces to d = lo[e1] + hi[e2] + db, y = tanh(d/2), row losses
loss_neg = softplus(y), loss_pos = softplus(y) - y.

v2 design changes vs v1:
  - replicated table build (no collective, no cross-core skew wait):
    every core loads all 16MB of features (f32->bf16 cast DMA, per-b
    pipelined), projects to per-branch tables via PE, writes bf16
    tables to DRAM, and broadcasts them to SBUF per-b as soon as each
    b's chunk lands (overlaps PE with broadcast DMA).
  - branch-routed Q7 cores: cores 0-3 (partitions 0-63) process the
    row branch, cores 4-7 the col branch.  Each partition then only
    needs its own branch's 64KB table -> 8MB broadcast instead of 16MB.
  - 3 ap_gathers (e1, e3, e2) of num_idxs=3200 instead of 6 of 1600.
  - gather offsets are computed on the host and uploaded as int16.
  - softplus activation fuses the exp+ln chain (2 ACT ops per row set).

Table row (branch-local): rb(b, n) = b*2048 + (n%128)*16 + (n//128),
so core-computed psum columns land contiguously.
"""

import numpy as np

import concourse.bass as bass
import concourse.bacc as bacc
import concourse.mybir as mybir
import concourse.tile as tile
from concourse.bass_utils import run_bass_kernel_spmd

F32 = mybir.dt.float32
BF16 = mybir.dt.bfloat16
I16 = mybir.dt.int16

B, C, N = 8, 256, 2048
R = 100000
NCORES = 8
PAIRS = R // NCORES          # 12500 pairs per core per branch
P = 128
QC = 4                       # Q7 cores per branch half
PPQ = PAIRS // QC            # 3125 real pairs per Q7 core
NI = 3128                    # ap_gather num_idxs per core (3125 + 3 pad)
GK = 196                     # int16 idx words per channel (ceil(NI/16))
# wave-1: the first W1 slots of every core hold pairs whose endpoints all
# have b <= 5, so their gathers only read the table prefix written by the
# first 12 broadcasts and can start ~25us before the full table is ready.
W1 = 512
BPREF = 4                    # wave-1 table prefix covers b < BPREF
# e2 chunks; each chunk's chains hide under the next chunk's gather.
E2_CHUNKS = ((0, W1), (W1, 1440), (1952, 656), (2608, 520))
TROWS = B * N                # 16384 rows per branch table
PADSLOT = 3126               # a known pad list position (>= PPQ, in the tail chunk)


def _emit_weight_prep(nc, const, psmall, w_row, w_col, b_row, b_col):
    """wp [128, 2kt*4m] bf16 (m = row_lo,row_hi,col_lo,col_hi) and
    dbp [128, 1] f32 = db/2 per partition (row for p<64, col for p>=64)."""
    wr_raw = const.tile([P, 8], F32, tag="wr_raw")
    wc_raw = const.tile([P, 8], F32, tag="wc_raw")
    nc.sync.dma_start(
        out=wr_raw[:].rearrange("p (s t) -> p s t", s=4),
        in_=w_row[:].rearrange("(s p) t -> p s t", p=P),
    )
    nc.sync.dma_start(
        out=wc_raw[:].rearrange("p (s t) -> p s t", s=4),
        in_=w_col[:].rearrange("(s p) t -> p s t", p=P),
    )
    wdiff_r = const.tile([P, 4], F32, tag="wdiff_r")
    wdiff_c = const.tile([P, 4], F32, tag="wdiff_c")
    nc.vector.tensor_tensor(
        out=wdiff_r[:], in0=wr_raw[:, 1::2], in1=wr_raw[:, 0::2],
        op=mybir.AluOpType.subtract,
    )
    nc.vector.tensor_tensor(
        out=wdiff_c[:], in0=wc_raw[:, 1::2], in1=wc_raw[:, 0::2],
        op=mybir.AluOpType.subtract,
    )
    wp = const.tile([P, 8], BF16, tag="wp")
    nc.vector.tensor_copy(out=wp[:, 0:8:4], in_=wdiff_r[:, 0:2])
    nc.vector.tensor_copy(out=wp[:, 1:8:4], in_=wdiff_r[:, 2:4])
    nc.vector.tensor_copy(out=wp[:, 2:8:4], in_=wdiff_c[:, 0:2])
    nc.vector.tensor_copy(out=wp[:, 3:8:4], in_=wdiff_c[:, 2:4])

    br_raw = const.tile([1, 2], F32, tag="br_raw")
    bc_raw = const.tile([1, 2], F32, tag="bc_raw")
    nc.sync.dma_start(out=br_raw[:], in_=b_row[:])
    nc.sync.dma_start(out=bc_raw[:], in_=b_col[:])
    db_rc = const.tile([1, 2], F32, tag="db_rc")
    nc.vector.tensor_tensor(
        out=db_rc[:, 0:1], in0=br_raw[:, 1:2], in1=br_raw[:, 0:1],
        op=mybir.AluOpType.subtract,
    )
    nc.vector.tensor_tensor(
        out=db_rc[:, 1:2], in0=bc_raw[:, 1:2], in1=bc_raw[:, 0:1],
        op=mybir.AluOpType.subtract,
    )
    # broadcast db/2 to all partitions via a 0.5-valued ones row
    half_row = const.tile([1, P], F32, tag="half_row")
    nc.vector.memset(half_row[:], 0.5)
    db_psum = psmall.tile([P, 2], F32, tag="db_psum")
    nc.tensor.matmul(
        db_psum[:], lhsT=half_row[:], rhs=db_rc[:], start=True, stop=True,
    )
    dbp = const.tile([P, 1], F32, tag="dbp")
    nc.vector.tensor_copy(out=dbp[0:64, :], in_=db_psum[0:64, 0:1])
    nc.vector.tensor_copy(out=dbp[64:P, :], in_=db_psum[64:P, 1:2])
    return wp, dbp


def _build_nc():
    nc = bacc.Bacc()

    feats = nc.declare_dram_parameter("feats", [B, C, N], F32, isOutput=False)
    w_row = nc.declare_dram_parameter("w_row", [2 * C, 2], F32, isOutput=False)
    w_col = nc.declare_dram_parameter("w_col", [2 * C, 2], F32, isOutput=False)
    b_row = nc.declare_dram_parameter("b_row", [1, 2], F32, isOutput=False)
    b_col = nc.declare_dram_parameter("b_col", [1, 2], F32, isOutput=False)
    idx = nc.declare_dram_parameter("idx", [P, 3 * GK], I16, isOutput=False)
    partial = nc.declare_dram_parameter("partial", [1], F32, isOutput=True)

    # t16[b, branch] = 2048 x (lo,hi) bf16 rows, row (n%128)*16 + n//128
    t16 = nc.dram_tensor("t16", [B, 2, N, 2], BF16)

    with tile.TileContext(nc) as tc:
        with (
            tc.tile_pool(name="const", bufs=1) as const,
            tc.tile_pool(name="fbpool", bufs=2) as fbpool,
            tc.tile_pool(name="chunkp", bufs=2) as chunkp,
            tc.tile_pool(name="work", bufs=2) as work,
            tc.tile_pool(name="psum", bufs=2, space="PSUM") as psum,
            tc.tile_pool(name="psmall", bufs=1, space="PSUM") as psmall,
        ):
            wp, dbp = _emit_weight_prep(nc, const, psmall, w_row, w_col,
                                        b_row, b_col)
            idx_sb = const.tile([P, 3 * GK], I16, tag="idx_sb")
            nc.sync.dma_start(out=idx_sb[:], in_=idx[:])

            # branch-split per-partition table: p<64 row table, p>=64 col
            table_sb = const.tile([P, TROWS * 2], BF16, tag="table_sb")

            # feats loads: cast-DMA stream on the gpsimd (SWDGE) queue;
            # chunk writes + broadcasts ride the sync/scalar queues so no
            # compute-dependent stall sits in front of a load.
            sbcs = []
            for b in range(B):
                fb = fbpool.tile([P, 2 * N], BF16, tag="fb", bufs=2)
                if b < B - 2:
                    nc.gpsimd.dma_start(
                        out=fb[:].rearrange("p (kt n) -> p kt n", kt=2),
                        in_=feats[b].rearrange("(kt p) n -> p kt n", p=P),
                    )
                else:
                    # last two b's ride the HWDGE queues as f32 + DVE cast
                    # so the gpsimd cast stream finishes ~20us earlier
                    for kt, eng in ((0, nc.sync), (1, nc.scalar)):
                        fb32 = fbpool.tile([P, N], F32, tag="fb32", bufs=2)
                        eng.dma_start(
                            out=fb32[:],
                            in_=feats[b, kt * P:(kt + 1) * P],
                        )
                        nc.vector.tensor_copy(
                            out=fb[:, kt * N:(kt + 1) * N], in_=fb32[:])
                pt = psum.tile([P, 64], F32, tag="pt", bufs=2)
                for blk in range(16):
                    nc.tensor.matmul(
                        pt[:, blk * 4:(blk + 1) * 4],
                        lhsT=fb[:, blk * P:(blk + 1) * P],
                        rhs=wp[:, 0:4], start=True, stop=False,
                    )
                    nc.tensor.matmul(
                        pt[:, blk * 4:(blk + 1) * 4],
                        lhsT=fb[:, N + blk * P:N + (blk + 1) * P],
                        rhs=wp[:, 4:8], start=False, stop=True,
                    )
                sbc = chunkp.tile([P, 64], BF16, tag="sbc", bufs=8)
                ptv = pt[:].rearrange("p (blk m) -> p blk m", m=4)
                nc.vector.tensor_copy(
                    out=sbc[:, 0:32].rearrange("p (blk t) -> p blk t", t=2),
                    in_=ptv[:, :, 0:2],
                )
                nc.vector.tensor_copy(
                    out=sbc[:, 32:64].rearrange("p (blk t) -> p blk t", t=2),
                    in_=ptv[:, :, 2:4],
                )
                sbcs.append(sbc)
            # table chunk writes trail on the sync queue
            wrs = []
            for b in range(B):
                for h in (0, 1):
                    w_dma = nc.sync.dma_start(
                        out=t16[b, h].rearrange("(q blk) t -> q blk t", q=P),
                        in_=sbcs[b][:, 32 * h:32 * (h + 1)].rearrange(
                            "p (blk t) -> p blk t", t=2),
                    )
                    wrs.append((b, h, w_dma))
            # broadcasts trail on sync (h=0) and scalar (h=1)
            bcast_eng = {0: nc.scalar, 1: nc.scalar}
            for b, h, w_dma in wrs:
                bc = bcast_eng[h].dma_start(
                    out=table_sb[64 * h:64 * (h + 1),
                                 b * 2 * N:(b + 1) * 2 * N],
                    in_=t16[b, h].rearrange(
                        "r t -> (r t)").partition_broadcast(64),
                )
                tile.add_dep_helper(bc.ins, w_dma.ins, sync=True,
                                    reason="broadcast after chunk write")

            # gathers: wave-1 reads only the b<BPREF table prefix
            table_ap = table_sb[:].rearrange("p (e d) -> p e d", d=2)
            pref_ap = table_sb[:, 0:BPREF * 2 * N].rearrange(
                "p (e d) -> p e d", d=2)

            def gather(s, f0w=0, ni=NI, pref=False):
                g_t = work.tile([P, ni, 2], BF16, tag=f"g{s}_{f0w}", bufs=1)
                nw = (ni + 15) // 16
                nc.gpsimd.ap_gather(
                    out_ap=g_t[:],
                    in_ap=pref_ap if pref else table_ap,
                    idxs_ap=idx_sb[:, s * GK + f0w:s * GK + f0w + nw],
                    channels=P,
                    num_elems=BPREF * N if pref else TROWS,
                    d=2, num_idxs=ni,
                )
                return g_t

            acc = const.tile([P, 14], F32, tag="acc")
            bias_one = const.tile([P, 1], F32, tag="bias_one")
            nc.vector.memset(bias_one[:], 1.0)


            def d_add(in0, in1, ni):
                d_t = work.tile([P, ni], BF16, tag=f"d{ni}", bufs=2)
                nc.vector.tensor_tensor(
                    out=d_t[:], in0=in0, in1=in1, op=mybir.AluOpType.add,
                )
                return d_t

            def tanh_op(nm, d_t, ni, acc_y=None):
                y_t = work.tile([P, ni], BF16, tag=f"y{nm}", bufs=1)
                kw = {}
                if acc_y is not None:
                    kw["accum_out"] = acc_y
                nc.scalar.activation(
                    out=y_t[:], in_=d_t[:],
                    func=mybir.ActivationFunctionType.Tanh,
                    bias=dbp[:, 0:1], scale=0.5, **kw,
                )
                return y_t

            def expln_op(nm, y_t, ni, acc_l, e_t):
                # e overwrites the (dead) d tile of the same chain
                nc.scalar.activation(
                    out=e_t[:], in_=y_t[:],
                    func=mybir.ActivationFunctionType.Exp,
                )
                l_t = work.tile([P, ni], BF16, tag=f"l{nm}", bufs=1)
                nc.scalar.activation(
                    out=l_t[:], in_=e_t[:],
                    func=mybir.ActivationFunctionType.Ln,
                    bias=bias_one[:, 0:1], scale=1.0,
                    accum_out=acc_l,
                )
                return l_t

            NI2 = NI - W1
            g1a = gather(0, 0, W1, pref=True)
            g3a = gather(2, 0, W1, pref=True)

            def e2_chunk(ci, c0, cn, g1t, g3t, o0):
                g2 = gather(1, c0 // 16, cn, pref=(ci == 0))
                d_pos_c = d_add(g1t[:, c0 - o0:c0 - o0 + cn, 0],
                                g2[:, :, 1], cn)
                d_negB_c = d_add(g3t[:, c0 - o0:c0 - o0 + cn, 0],
                                 g2[:, :, 1], cn)
                y_pos_c = tanh_op(f"pos{ci}", d_pos_c, cn,
                                  acc_y=acc[:, 10 + ci:11 + ci])
                y_negB_c = tanh_op(f"negB{ci}", d_negB_c, cn)
                l_pos_c = expln_op(f"pos{ci}", y_pos_c, cn,
                                   acc[:, 2 + 2 * ci:3 + 2 * ci], d_pos_c)
                expln_op(f"negB{ci}", y_negB_c, cn,
                         acc[:, 3 + 2 * ci:4 + 2 * ci], d_negB_c)
                return y_pos_c, l_pos_c

            # wave-1 e2 chunk + its chains run while the full table finishes
            e2_chunk(0, 0, W1, g1a, g3a, 0)
            d_negA1 = d_add(g3a[:, :, 0], g1a[:, :, 1], W1)
            y_negA1 = tanh_op("negA1", d_negA1, W1)
            expln_op("negA1", y_negA1, W1, acc[:, 0:1], d_negA1)
            # wave-2 full-table gathers
            g1b = gather(0, W1 // 16, NI2)
            g3b = gather(2, W1 // 16, NI2)
            d_negA2 = d_add(g3b[:, :, 0], g1b[:, :, 1], NI2)
            y_negA2 = tanh_op("negA2", d_negA2, NI2)
            expln_op("negA2", y_negA2, NI2, acc[:, 1:2], d_negA2)
            y_pos = l_pos = None
            for ci, (c0, cn) in enumerate(E2_CHUNKS[1:], start=1):
                y_pos, l_pos = e2_chunk(ci, c0, cn, g1b, g3b, W1)


            # total = sum(L) - sum(y_pos), scaled by 1/(3R*16)
            t_l = const.tile([P, 1], F32, tag="t_l")
            nc.vector.tensor_reduce(
                out=t_l[:], in_=acc[:, 0:10], axis=mybir.AxisListType.X,
                op=mybir.AluOpType.add,
            )
            t_y = const.tile([P, 1], F32, tag="t_y")
            nc.vector.tensor_reduce(
                out=t_y[:], in_=acc[:, 10:14], axis=mybir.AxisListType.X,
                op=mybir.AluOpType.add,
            )
            total = const.tile([P, 1], F32, tag="total")
            nc.vector.tensor_tensor(
                out=total[:], in0=t_l[:], in1=t_y[:],
                op=mybir.AluOpType.subtract,
            )
            scale_vec = const.tile([P, 1], F32, tag="scale_vec")
            nc.vector.memset(scale_vec[:], 1.0 / (3.0 * R * 16.0))
            out_psum = psmall.tile([1, 1], F32, tag="out_psum")
            nc.tensor.matmul(
                out_psum[:], lhsT=total[:], rhs=scale_vec[:],
                start=True, stop=True,
            )
            out_sb = const.tile([1, 1], F32, tag="out_sb")
            nc.vector.tensor_copy(out=out_sb[:], in_=out_psum[:])

            # pad correction: each branch half has 4 cores x 75 pad pairs,
            # all reading table row 0; per pad pair the pollution is
            # 3*L - y, totalling (3L - y)*300/(3R) = 0.003*L - 0.001*y.
            corr = const.tile([1, 4], F32, tag="corr")
            ps = PADSLOT - E2_CHUNKS[-1][0]  # pad slot within last chunk
            for h in (0, 1):
                nc.vector.tensor_copy(
                    out=corr[0:1, h:h + 1],
                    in_=l_pos[64 * h:64 * h + 1, ps:ps + 1])
                nc.vector.tensor_copy(
                    out=corr[0:1, 2 + h:3 + h],
                    in_=y_pos[64 * h:64 * h + 1, ps:ps + 1])
            cs = const.tile([1, 2], F32, tag="cs")
            nc.vector.tensor_tensor(out=cs[0:1, 0:1], in0=corr[0:1, 0:1],
                                    in1=corr[0:1, 1:2],
                                    op=mybir.AluOpType.add)
            nc.vector.tensor_tensor(out=cs[0:1, 1:2], in0=corr[0:1, 2:3],
                                    in1=corr[0:1, 3:4],
                                    op=mybir.AluOpType.add)
            # 3 pads/core x 4 cores = 12 pad pairs per branch:
            # pollution = (3L - y) * 12 / (3R)
            cs2 = const.tile([1, 2], F32, tag="cs2")
            nc.vector.tensor_scalar(out=cs2[0:1, 0:1], in0=cs[0:1, 0:1],
                                    scalar1=3.0 * 12 / (3.0 * R),
                                    scalar2=None,
                                    op0=mybir.AluOpType.mult)
            nc.vector.tensor_scalar(out=cs2[0:1, 1:2], in0=cs[0:1, 1:2],
                                    scalar1=12 / (3.0 * R), scalar2=None,
                                    op0=mybir.AluOpType.mult)
            out2 = const.tile([1, 1], F32, tag="out2")
            nc.vector.tensor_tensor(out=out2[:], in0=out_sb[:],
                                    in1=cs2[0:1, 0:1],
                                    op=mybir.AluOpType.subtract)
            nc.vector.tensor_tensor(out=out2[:], in0=out2[:],
                                    in1=cs2[0:1, 1:2],
                                    op=mybir.AluOpType.add)
            nc.sync.dma_start(out=partial[:], in_=out2[0, :])
    return nc


_NC_CACHE = {}


def _get_nc():
    if "v2" not in _NC_CACHE:
        nc = _build_nc()
        nc.finalize()
        _NC_CACHE["v2"] = nc
    return _NC_CACHE["v2"]


def _pack_core_inputs(inputs, core):
    """Host-side: compute branch-local int16 table offsets and arrange
    them in the (Q7-core wrapped) ap_gather index layout.

    Slot (p, s*GK + F): half h = p//64, core c = (p%64)//16, lane
    u = p%16, list pos j = F*16 + u; pair = c*PPQ + j for j < PPQ,
    else pad (offset 0)."""
    base = core * PAIRS
    sl = slice(base, base + PAIRS)
    branches = [
        (inputs["row_pos_b"][sl], inputs["row_pos_i"][sl],
         inputs["row_pos_j"][sl], inputs["row_neg_b"][sl],
         inputs["row_neg_i"][sl]),
        (inputs["col_pos_b"][sl], inputs["col_pos_i"][sl],
         inputs["col_pos_j"][sl], inputs["col_neg_b"][sl],
         inputs["col_neg_i"][sl]),
    ]

    def off(b, n):
        b = np.asarray(b, np.int64)
        n = np.asarray(n, np.int64)
        return b * 2048 + (n & 127) * 16 + (n >> 7)

    arr = np.zeros((P, 3 * GK), np.int16)
    u = np.arange(16)[:, None]
    F = np.arange(GK)[None, :]
    j = F * 16 + u                      # [16, GK] list positions
    valid = j < PPQ
    jc = np.minimum(j, PPQ - 1)
    for h, (pb, pi, pj, nb, ni_) in enumerate(branches):
        offs = (off(pb, pi), off(pb, pj), off(nb, ni_))
        w1m = (np.asarray(pb) < BPREF) & (np.asarray(nb) < BPREF)
        for c in range(QC):
            lo = c * PPQ
            w1c = np.nonzero(w1m[lo:lo + PPQ])[0]
            assert len(w1c) >= W1, len(w1c)
            rest = np.concatenate(
                [w1c[W1:], np.nonzero(~w1m[lo:lo + PPQ])[0]])
            perm = np.concatenate([w1c[:W1], rest]) + lo
            for s in range(3):
                o = offs[s][perm]
                vals = np.where(valid, o[jc], 0)
                arr[h * 64 + c * 16:h * 64 + (c + 1) * 16,
                    s * GK:(s + 1) * GK] = vals.astype(np.int16)

    im = {
        "w_row": np.ascontiguousarray(np.asarray(inputs["W_row"], np.float32)),
        "w_col": np.ascontiguousarray(np.asarray(inputs["W_col"], np.float32)),
        "b_row": np.ascontiguousarray(
            np.asarray(inputs["b_row"], np.float32).reshape(1, 2)),
        "b_col": np.ascontiguousarray(
            np.asarray(inputs["b_col"], np.float32).reshape(1, 2)),
        "idx": arr,
        "feats": np.ascontiguousarray(
            np.asarray(inputs["all_features"], np.float32)),
    }
    return im


def run(inputs, trace=False):
    nc = _get_nc()
    in_maps = [_pack_core_inputs(inputs, c) for c in range(NCORES)]
    res = run_bass_kernel_spmd(nc, in_maps, list(range(NCORES)), trace=trace)
    partials = np.array(
        [res.results[c]["partial"][0] for c in range(NCORES)], np.float32
    )
    out = np.array([partials.sum()], np.float32)
    return out, res


def kernel(**inputs):
    out, _ = run(inputs, trace=False)
    return out
